# revision 39
# baseline (speedup 1.0000x reference)
"""Trainium2 Bass kernel for the Basicgate multivoxel attention module.

The chain voxel-features -> attention logit is linear, so it collapses:

  logit(h,w) = sum_k T[k, h+dy_k-1, w+dx_k-1]            (point terms)
             + sum_k S[k] * gated(h+dy_k-1, w+dx_k-1)    (gated 3x3)
             + edge-constant terms                        (biases + padding)
  out = img * sigmoid(logit + sp_b)

where per point p of set i at cell (hp,wp): T[:, hp, wp] += B_i @ x_p with
B0 = V@W2@W0 (9,35), B1 = V@W2@W1 (9,67), B2 = V@W2 (9,131), x_p the
concat(feat, coord) vector, V (9,131) the 3x3 conv taps; gated = w3.img + b3
per pixel; S[k] = sum_c V[k,c].

Sharding: H split across 8 cores (32 rows each + 1 halo row per side).

Key trick vs. a scatter-based design: the host places each point's x vector
at a dense cell-indexed column (cell = h*768 + (w+1), zero columns for unhit
cells; cells are unique per set so no collision). Then T_tot = sum_i B_i@x_i
is computed entirely on the PE in 128-cell chunks whose output partitions ARE
the cell layout stage D wants: T_sb[p, b*9+k] = T[cell=b*128+p, k], i.e.
partitions = w mod 128, free = (h*6 + w//128)*9 + k. No scatter-add DMA, no
DRAM scratch, no zeroing, no strided readback.

Device pipeline per core:
  B. per 12-block group: load xA/xB (bf16) chunks, 2 accumulating PE matmuls
     per 128-cell block (sets 0+1+2-tail stacked to 105 rows / set2 main 128
     rows) -> psum [128, 108]; ACT-copy to T_sb.
  C. stream img (34 rows x 2) through PE against w3 -> gated map (34,706)
  D. DVE: U[m][d][p,r] = sum_dy T_sb taps (3 strided slices); PE: logit in
     PSUM [32, 704] = 3 gated taps (row layout) + per-wtile shift matmuls
     (moving = host shift matrices) incl. 2-partition boundary fixes; DVE
     column edge fixes; sigmoid on ACT with per-row bias.
  E. per 2 rows: PE-replicate att (bf16) to 128 partitions, re-stream img,
     DVE multiply, DMA out.
"""

import numpy as np
import ml_dtypes

# ---- problem constants (hardcoded per contract) ----
C_IMG = 256
H, W = 256, 704
CH = [32, 64, 128]
COUT = 131
N_CORES = 8
R = 32            # owned rows per core
L = 34            # local rows incl 1-row halo each side
WP = W + 2        # padded width (706 used, stored stride 768)
WS = 768          # padded-width storage stride = 6*128
NT = 6            # w tiles of 128
BLOCKS = L * NT   # 204 cell blocks of 128
CELLS = BLOCKS * 128  # 26112 dense cell columns
GRP = 12          # blocks per psum group
NGRP = BLOCKS // GRP  # 17
KA = CH[0] + 3 + CH[1] + 3 + 3   # 105 = set0(35) + set1(67) + set2 tail(3)
KB = 128                          # set2 main rows

BF16 = ml_dtypes.bfloat16
LAST_RESULT = None  # stash of BassKernelResults for the test harness


def _fold_weights(inputs):
    f8 = np.float64
    W0 = inputs["rd0_w"][:, :, 0, 0].astype(f8)   # (131, 35)
    W1 = inputs["rd1_w"][:, :, 0, 0].astype(f8)   # (131, 67)
    W2 = inputs["rd2_w"][:, :, 0, 0].astype(f8)   # (131, 131)
    w3 = inputs["rd3_w"][0, :, 0, 0].astype(f8)   # (256,)
    b0 = inputs["rd0_b"].astype(f8)
    b1 = inputs["rd1_b"].astype(f8)
    b2 = inputs["rd2_b"].astype(f8)
    b3 = float(inputs["rd3_b"][0])
    spb = float(inputs["sp_b"][0])
    # V[k=dy*3+dx, c] = sp_w[0, c, dy, dx]
    V = inputs["sp_w"][0].astype(f8).transpose(1, 2, 0).reshape(9, COUT)
    B = [V @ (W2 @ W0), V @ (W2 @ W1), V @ W2]
    S = V.sum(axis=1)                # (9,)
    # gated-map b3 has the same padding support as the bias constants:
    # fold it into cc so gmap carries only the raw w3.img term
    cc = V @ (W2 @ (b0 + b1) + b2) + b3 * S   # (9,)
    return dict(B=B, cc=cc, S=S, C_all=float(cc.sum()),
                w3=w3, b3=b3, spb=spb)


def _build_program(b3):
    import concourse.bacc as bacc
    import concourse.mybir as mybir
    import concourse.tile as tile

    f32 = mybir.dt.float32
    bf16 = mybir.dt.bfloat16
    Alu = mybir.AluOpType
    Act = mybir.ActivationFunctionType

    nc = bacc.Bacc("TRN2", target_bir_lowering=False, debug=False,
                   num_devices=N_CORES)

    img = nc.dram_tensor("img", [C_IMG, L, W], bf16, kind="ExternalInput").ap()
    xa = nc.dram_tensor("xa", [KA, CELLS], bf16, kind="ExternalInput").ap()
    xb = nc.dram_tensor("xb", [KB, CELLS], bf16, kind="ExternalInput").ap()
    bta_d = nc.dram_tensor("bta", [KA, 9], bf16, kind="ExternalInput").ap()
    btb_d = nc.dram_tensor("btb", [KB, 9], bf16, kind="ExternalInput").ap()
    w3d = nc.dram_tensor("w3", [C_IMG], f32, kind="ExternalInput").ap()
    # shift matrices: [128, 3*128 + 3] = Sh0 | Sh1 | Sh2 | Shb1(1) | Shb2(2)
    shd = nc.dram_tensor("shmats", [128, 387], bf16, kind="ExternalInput").ap()
    emgd = nc.dram_tensor("emg", [L, 96], bf16, kind="ExternalInput").ap()
    rowfixd = nc.dram_tensor("rowfix", [R, 1], f32, kind="ExternalInput").ap()
    colfix0d = nc.dram_tensor("colfix0", [R, 1], f32, kind="ExternalInput").ap()
    colfix1d = nc.dram_tensor("colfix1", [R, 1], f32, kind="ExternalInput").ap()
    out = nc.dram_tensor("out", [C_IMG, R, W], f32, kind="ExternalOutput").ap()

    with tile.TileContext(nc) as tc:
        with (
            tc.tile_pool(name="persist", bufs=1) as pp,
            tc.tile_pool(name="work", bufs=3) as wp,
            tc.tile_pool(name="imgp", bufs=3) as ip,
            tc.tile_pool(name="psum", bufs=1, space="PSUM") as psp,
            tc.tile_pool(name="pg", bufs=2, space="PSUM") as pgp,
            tc.tile_pool(name="plg", bufs=1, space="PSUM") as plgp,
            tc.tile_pool(name="ppts", bufs=1, space="PSUM") as psb,
        ):
            # ---- persistent small tensors ----
            bta_t = pp.tile([KA, 9], bf16, tag="bta")
            nc.sync.dma_start(out=bta_t[:], in_=bta_d[:])
            btb_t = pp.tile([KB, 9], bf16, tag="btb")
            nc.sync.dma_start(out=btb_t[:], in_=btb_d[:])
            w3f_t = pp.tile([C_IMG // 2, 2], f32, tag="w3f")
            nc.sync.dma_start(out=w3f_t[:],
                              in_=w3d[:].rearrange("(hh c) -> c hh", hh=2))
            w3_t = pp.tile([C_IMG // 2, 2], bf16, tag="w3")
            nc.vector.tensor_copy(out=w3_t[:], in_=w3f_t[:])
            sh_t = pp.tile([128, 387], bf16, tag="shmats")
            nc.sync.dma_start(out=sh_t[:], in_=shd[:])
            emg_t = pp.tile([L, 96], bf16, tag="emg")
            nc.sync.dma_start(out=emg_t[:], in_=emgd[:])
            ones_t = pp.tile([1, 128], bf16, tag="ones")
            nc.vector.memset(ones_t[:], 1.0)
            RB = R // 4
            rowfix_b, colfix0_b, colfix1_b = [], [], []
            for b in range(4):
                rt = pp.tile([RB, 1], f32, tag=f"rowfix{b}", name=f"rowfix{b}")
                nc.sync.dma_start(out=rt[:], in_=rowfixd[b * RB:(b + 1) * RB, :])
                rowfix_b.append(rt)
                c0t = pp.tile([RB, 1], f32, tag=f"colfix0{b}", name=f"cf0{b}")
                nc.sync.dma_start(out=c0t[:],
                                  in_=colfix0d[b * RB:(b + 1) * RB, :])
                colfix0_b.append(c0t)
                c1t = pp.tile([RB, 1], f32, tag=f"colfix1{b}", name=f"cf1{b}")
                nc.sync.dma_start(out=c1t[:],
                                  in_=colfix1d[b * RB:(b + 1) * RB, :])
                colfix1_b.append(c1t)

            # preload the sigmoid ACT table off the critical path
            warm = pp.tile([1, 2], f32, tag="warm")
            nc.vector.memset(warm[:], 0.0)
            nc.scalar.activation(warm[:, 0:1], warm[:, 1:2], Act.Sigmoid,
                                 bias=0.0, scale=1.0)

            gmap = pp.tile([L, WP], bf16, tag="gmap")
            nc.vector.memset(gmap[:], 0.0)
            T_sb = pp.tile([128, BLOCKS * 9], bf16, tag="Tsb")
            att_b = [pp.tile([RB, W], f32, tag=f"att{b}", name=f"att{b}")
                     for b in range(4)]
            attbf_b = [pp.tile([RB, W], bf16, tag=f"attbf{b}",
                               name=f"attbf{b}") for b in range(4)]
            # persistent bf16 image cache: [2 c-halves][128, L*W]
            img_bf = [pp.tile([128, L * W], bf16, tag=f"imgbf{hh}",
                              name=f"imgbf{hh}")
                      for hh in range(2)]

            # ---- two row-phases: each runs its half of stage B groups and
            # stage C rows, then assembles attention for its 16-row band and
            # streams the output writes overlapped with the next phase ----
            T3 = T_sb[:].rearrange("p (h x) -> p h x", x=NT * 9)
            segs = ((0, 512), (512, 192))
            BR = R // 4
            GRP_PH = (range(0, 5), range(5, 9), range(9, 13),
                      range(13, NGRP))
            RC_PH = (range(0, 5), range(5, 9), range(9, 13),
                     range(13, L // 2))
            for ph in range(4):
                b0r = ph * BR

                # -- stage B groups + stage C rows, interleaved so stage C
                # matmuls don't queue behind the whole phase's point pairs --
                for g, rc in zip(GRP_PH[ph], RC_PH[ph]):
                    c0 = g * GRP * 128
                    xat = wp.tile([KA, GRP * 128], bf16, tag="xa")
                    nc.sync.dma_start(out=xat[:],
                                      in_=xa[:, c0:c0 + GRP * 128])
                    xbt = wp.tile([KB, GRP * 128], bf16, tag="xb")
                    nc.sync.dma_start(out=xbt[:],
                                      in_=xb[:, c0:c0 + GRP * 128])
                    tpsum = psb.tile([128, GRP * 9], f32, tag="pts")
                    for bl in range(GRP):
                        po = bl * 9
                        cx = bl * 128
                        nc.tensor.matmul(tpsum[:, po:po + 9],
                                         xbt[:, cx:cx + 128], btb_t[:],
                                         start=True, stop=False)
                        nc.tensor.matmul(tpsum[:, po:po + 9],
                                         xat[:, cx:cx + 128], bta_t[:],
                                         start=False, stop=True)
                    nc.scalar.copy(
                        out=T_sb[:, g * GRP * 9:(g + 1) * GRP * 9],
                        in_=tpsum[:])
                    imgt = []
                    for hh in range(2):
                        ib = img_bf[hh][:, 2 * rc * W:(2 * rc + 2) * W]
                        nc.scalar.dma_start(
                            out=ib,
                            in_=img[hh * 128:(hh + 1) * 128,
                                    2 * rc:2 * rc + 2, :].rearrange(
                                        "c r w -> c (r w)"))
                        imgt.append(ib)
                    gstage = wp.tile([1, 2 * W], bf16, tag="gstage")
                    for off, n in ((0, 512), (512, 512), (1024, 384)):
                        gp = pgp.tile([1, 512], f32, tag="gp")
                        for hh in range(2):
                            nc.tensor.matmul(
                                gp[:, 0:n],
                                w3_t[:, hh:hh + 1],
                                imgt[hh][:, off:off + n],
                                start=(hh == 0), stop=(hh == 1))
                        nc.scalar.copy(out=gstage[:, off:off + n],
                                       in_=gp[:, 0:n])
                    for r01 in range(2):
                        nc.scalar.dma_start(
                            out=gmap[2 * rc + r01:2 * rc + r01 + 1, 1:1 + W],
                            in_=gstage[:, r01 * W:(r01 + 1) * W])

                # -- dy-reduce for this band (GpSimd) --
                U = [[None] * 3 for _ in range(NT)]
                for m in range(NT):
                    for d in range(3):
                        u = pp.tile([128, BR], bf16, tag=f"u{m}{d}{ph}",
                                    name=f"u{m}{d}{ph}")
                        o = lambda dy: m * 9 + 3 * dy + d
                        nc.gpsimd.tensor_tensor(
                            out=u[:], in0=T3[:, b0r:b0r + BR, o(0)],
                            in1=T3[:, b0r + 1:b0r + 1 + BR, o(1)], op=Alu.add)
                        nc.gpsimd.tensor_tensor(
                            out=u[:], in0=u[:],
                            in1=T3[:, b0r + 2:b0r + 2 + BR, o(2)],
                            op=Alu.add)
                        U[m][d] = u

                # -- logit assembly for this band on PE --
                # band 0 contracts over only its 18 gmap rows (rows 18+ not
                # written yet); band 1 uses all 34 (rows 0..15 have zero
                # weights in its emg column slice). Matmul operand slices
                # must start at partition 0.
                hc = min(b0r + BR + 2, L)
                ge = emg_t[0:hc, :]
                gm = gmap[0:hc, :]
                lg = plgp.tile([BR, W], f32, tag="lg")
                for gi, (off, n) in enumerate(segs):
                    nc.tensor.matmul(
                        lg[:, off:off + n],
                        ge[:, 0 + b0r:0 + b0r + BR],
                        gm[:, 0:W][:, off:off + n], start=True, stop=False)
                    nc.tensor.matmul(
                        lg[:, off:off + n],
                        ge[:, 32 + b0r:32 + b0r + BR],
                        gm[:, 1:1 + W][:, off:off + n],
                        start=False, stop=False)
                for m in range(NT):
                    mc = m * 128
                    wm = 128 if m < NT - 1 else W - 128 * (NT - 1)
                    for d in range(3):
                        nc.tensor.matmul(
                            lg[:, mc:mc + wm], U[m][d][:],
                            sh_t[:, d * 128:d * 128 + wm],
                            start=False, stop=False)
                    if m < NT - 1:
                        nc.tensor.matmul(
                            lg[:, mc + 127:mc + 128], U[m + 1][1][:],
                            sh_t[:, 384:385], start=False, stop=False)
                        nc.tensor.matmul(
                            lg[:, mc + 126:mc + 128], U[m + 1][2][:],
                            sh_t[:, 385:387], start=False, stop=False)
                for gi, (off, n) in enumerate(segs):
                    nc.tensor.matmul(
                        lg[:, off:off + n],
                        ge[:, 64 + b0r:64 + b0r + BR],
                        gm[:, 2:2 + W][:, off:off + n],
                        start=False, stop=True)
                nc.vector.tensor_tensor(
                    out=lg[:, 0:1], in0=lg[:, 0:1],
                    in1=colfix0_b[ph][:, 0:1], op=Alu.add)
                nc.vector.tensor_tensor(
                    out=lg[:, W - 1:W], in0=lg[:, W - 1:W],
                    in1=colfix1_b[ph][:, 0:1], op=Alu.add)
                # rowfix carries C_all + sp_b + row-edge constants
                nc.scalar.activation(att_b[ph][:], lg[:], Act.Sigmoid,
                                     bias=rowfix_b[ph][:, 0:1], scale=1.0)
                nc.vector.tensor_copy(out=attbf_b[ph][:], in_=att_b[ph][:])

                # -- broadcast multiply + store for this band --
                for rc in range(ph * (BR // 2), (ph + 1) * (BR // 2)):
                    a1p = wp.tile([1, 2 * W], bf16, tag="a1p")
                    for r01 in range(2):
                        rl = 2 * rc + r01 - b0r
                        nc.scalar.dma_start(
                            out=a1p[:, r01 * W:(r01 + 1) * W],
                            in_=attbf_b[ph][rl:rl + 1, :])
                    attb = psp.tile([128, 2 * W], f32, tag="big")
                    for off, n in ((0, 512), (512, 512), (1024, 384)):
                        nc.tensor.matmul(
                            attb[:, off:off + n],
                            ones_t[:],
                            a1p[:, off:off + n],
                            start=True, stop=True)
                    attb_sb = wp.tile([128, 2 * W], bf16, tag="attbsb")
                    nc.scalar.copy(out=attb_sb[:], in_=attb[:])
                    for hh in range(2):
                        ib = img_bf[hh][:, (2 * rc + 1) * W:(2 * rc + 3) * W]
                        ot = ip.tile([128, 2 * W], f32, tag="out")
                        nc.vector.tensor_tensor(out=ot[:], in0=ib,
                                                in1=attb_sb[:], op=Alu.mult)
                        nc.sync.dma_start(
                            out=out[hh * 128:(hh + 1) * 128,
                                    2 * rc:2 * rc + 2, :].rearrange(
                                        "c r w -> c (r w)"),
                            in_=ot[:])

    nc.compile()
    return nc


def _prepare(inputs):
    """Host-side fold + shard. Returns (b3, in_maps)."""
    fold = _fold_weights(inputs)
    cc, S = fold["cc"], fold["S"]

    grids = [np.asarray(inputs[f"img_grid_{i}"]) for i in range(3)]
    feats = [np.asarray(inputs[f"voxel_feat_{i}"]) for i in range(3)]
    coords = [np.asarray(inputs[f"voxel_coord_{i}"]) for i in range(3)]
    img_feat = np.asarray(inputs["img_feat"])

    # stacked (set0|set1|set2-tail) and set2-main folded B matrices, bf16
    bta = np.concatenate(
        [fold["B"][0].T, fold["B"][1].T, fold["B"][2].T[128:]], axis=0)
    btb = fold["B"][2].T[:128]

    # shift matrices Sh_d[c, f] = 1 iff c == f + d; boundary picks
    sh = np.zeros((128, 387), BF16)
    for d in range(3):
        for f in range(128 - d):
            sh[f + d, d * 128 + f] = 1.0
    sh[0, 384] = 1.0              # Shb1: col f=127 <- U_{m+1,1}[0]
    sh[0, 385] = 1.0              # Shb2: col f=126 <- U_{m+1,2}[0]
    sh[1, 386] = 1.0              # Shb2: col f=127 <- U_{m+1,2}[1]

    # gated tap matrices: emg[p+dy, dx*32+p] = S[dy*3+dx] (f32 base;
    # per-core rowmask is folded in before the bf16 cast)
    emg_base = np.zeros((L, 96), np.float32)
    for dx in range(3):
        for dy in range(3):
            for p in range(R):
                emg_base[p + dy, dx * 32 + p] = np.float32(S[dy * 3 + dx])

    # per-set x (channels x tokens), f32 -> bf16 once
    xs_bf = []
    for i in range(3):
        xs_bf.append(np.ascontiguousarray(np.concatenate(
            [feats[i], coords[i]], axis=1).T).astype(BF16))

    in_maps = []
    for c in range(N_CORES):
        lo = R * c - 1
        m = {}
        slab = np.zeros((C_IMG, L, W), BF16)
        g0, g1 = max(lo, 0), min(lo + L, H)
        slab[:, g0 - lo:g1 - lo, :] = img_feat[:, g0:g1, :].astype(BF16)
        m["img"] = slab

        xa = np.zeros((KA, CELLS), BF16)
        xb = np.zeros((KB, CELLS), BF16)
        offs = [0, CH[0] + 3, None]
        for i in range(3):
            rows = grids[i][:, 1]
            sel = np.nonzero((rows >= lo) & (rows < lo + L))[0]
            if not len(sel):
                continue
            hl = rows[sel].astype(np.int64) - lo
            wl = grids[i][sel, 0].astype(np.int64) + 1
            cell = hl * WS + wl
            xi = xs_bf[i][:, sel]
            if i < 2:
                xa[offs[i]:offs[i] + xi.shape[0], cell] = xi
            else:
                xb[:, cell] = xi[:128]
                xa[KA - 3:, cell] = xi[128:]
        m["xa"] = xa
        m["xb"] = xb
        m["bta"] = np.ascontiguousarray(bta).astype(BF16)
        m["btb"] = np.ascontiguousarray(btb).astype(BF16)
        m["w3"] = fold["w3"].astype(np.float32)
        m["shmats"] = sh
        rowmask = np.zeros((L, 1), np.float32)
        rowmask[g0 - lo:g1 - lo] = 1.0
        m["emg"] = (emg_base * rowmask).astype(BF16)
        # rowfix: C_all + sp_b + row-edge constants (used as sigmoid bias)
        rowfix = np.full((R, 1), fold["C_all"] + fold["spb"], np.float64)
        colfix0 = np.full((R, 1), -(cc[0] + cc[3] + cc[6]))
        colfix1 = np.full((R, 1), -(cc[2] + cc[5] + cc[8]))
        for hloc in range(R):
            g = R * c + hloc
            if g == 0:
                rowfix[hloc] += -(cc[0] + cc[1] + cc[2])
                colfix0[hloc] += cc[0]
                colfix1[hloc] += cc[2]
            if g == H - 1:
                rowfix[hloc] += -(cc[6] + cc[7] + cc[8])
                colfix0[hloc] += cc[6]
                colfix1[hloc] += cc[8]
        m["rowfix"] = rowfix.astype(np.float32)
        m["colfix0"] = colfix0.astype(np.float32)
        m["colfix1"] = colfix1.astype(np.float32)
        in_maps.append(m)
    return fold["b3"], in_maps


def kernel(**inputs):
    global LAST_RESULT
    from concourse.bass_utils import run_bass_kernel_spmd

    b3, in_maps = _prepare(inputs)
    nc = _build_program(b3)
    res = run_bass_kernel_spmd(nc, in_maps, core_ids=list(range(N_CORES)))
    LAST_RESULT = res
    out = np.concatenate(
        [res.results[c]["out"] for c in range(N_CORES)], axis=1)
    return np.ascontiguousarray(out.astype(np.float32))


# revision 41
# speedup vs baseline: 1.0720x; 1.0720x over previous
"""Trainium2 Bass kernel for the Basicgate multivoxel attention module.

The chain voxel-features -> attention logit is linear, so it collapses:

  logit(h,w) = sum_k T[k, h+dy_k-1, w+dx_k-1]            (point terms)
             + sum_k S[k] * gated(h+dy_k-1, w+dx_k-1)    (gated 3x3)
             + edge-constant terms                        (biases + padding)
  out = img * sigmoid(logit + sp_b)

where per point p of set i at cell (hp,wp): T[:, hp, wp] += B_i @ x_p with
B0 = V@W2@W0 (9,35), B1 = V@W2@W1 (9,67), B2 = V@W2 (9,131), x_p the
concat(feat, coord) vector, V (9,131) the 3x3 conv taps; gated = w3.img + b3
per pixel; S[k] = sum_c V[k,c].

Sharding: H split across 8 cores (32 rows each + 1 halo row per side).

Key trick vs. a scatter-based design: the host places each point's x vector
at a dense cell-indexed column (cell = h*768 + (w+1), zero columns for unhit
cells; cells are unique per set so no collision). Then T_tot = sum_i B_i@x_i
is computed entirely on the PE in 128-cell chunks whose output partitions ARE
the cell layout stage D wants: T_sb[p, b*9+k] = T[cell=b*128+p, k], i.e.
partitions = w mod 128, free = (h*6 + w//128)*9 + k. No scatter-add DMA, no
DRAM scratch, no zeroing, no strided readback.

Device pipeline per core:
  B. per 12-block group: load xA/xB (bf16) chunks, 2 accumulating PE matmuls
     per 128-cell block (sets 0+1+2-tail stacked to 105 rows / set2 main 128
     rows) -> psum [128, 108]; ACT-copy to T_sb.
  C. stream img (34 rows x 2) through PE against w3 -> gated map (34,706)
  D. DVE: U[m][d][p,r] = sum_dy T_sb taps (3 strided slices); PE: logit in
     PSUM [32, 704] = 3 gated taps (row layout) + per-wtile shift matmuls
     (moving = host shift matrices) incl. 2-partition boundary fixes; DVE
     column edge fixes; sigmoid on ACT with per-row bias.
  E. per 2 rows: PE-replicate att (bf16) to 128 partitions, re-stream img,
     DVE multiply, DMA out.
"""

import numpy as np
import ml_dtypes

# ---- problem constants (hardcoded per contract) ----
C_IMG = 256
H, W = 256, 704
CH = [32, 64, 128]
COUT = 131
N_CORES = 8
R = 32            # owned rows per core
L = 34            # local rows incl 1-row halo each side
WP = W + 2        # padded width (706 used, stored stride 768)
WS = 768          # padded-width storage stride = 6*128
NT = 6            # w tiles of 128
BLOCKS = L * NT   # 204 cell blocks of 128
CELLS = BLOCKS * 128  # 26112 dense cell columns
GRP = 12          # blocks per psum group
NGRP = BLOCKS // GRP  # 17
KA = CH[0] + 3 + CH[1] + 3 + 3   # 105 = set0(35) + set1(67) + set2 tail(3)
KB = 128                          # set2 main rows

BF16 = ml_dtypes.bfloat16
LAST_RESULT = None  # stash of BassKernelResults for the test harness


def _fold_weights(inputs):
    f8 = np.float64
    W0 = inputs["rd0_w"][:, :, 0, 0].astype(f8)   # (131, 35)
    W1 = inputs["rd1_w"][:, :, 0, 0].astype(f8)   # (131, 67)
    W2 = inputs["rd2_w"][:, :, 0, 0].astype(f8)   # (131, 131)
    w3 = inputs["rd3_w"][0, :, 0, 0].astype(f8)   # (256,)
    b0 = inputs["rd0_b"].astype(f8)
    b1 = inputs["rd1_b"].astype(f8)
    b2 = inputs["rd2_b"].astype(f8)
    b3 = float(inputs["rd3_b"][0])
    spb = float(inputs["sp_b"][0])
    # V[k=dy*3+dx, c] = sp_w[0, c, dy, dx]
    V = inputs["sp_w"][0].astype(f8).transpose(1, 2, 0).reshape(9, COUT)
    B = [V @ (W2 @ W0), V @ (W2 @ W1), V @ W2]
    S = V.sum(axis=1)                # (9,)
    # gated-map b3 has the same padding support as the bias constants:
    # fold it into cc so gmap carries only the raw w3.img term
    cc = V @ (W2 @ (b0 + b1) + b2) + b3 * S   # (9,)
    return dict(B=B, cc=cc, S=S, C_all=float(cc.sum()),
                w3=w3, b3=b3, spb=spb)


def _build_program(b3):
    import concourse.bacc as bacc
    import concourse.mybir as mybir
    import concourse.tile as tile

    f32 = mybir.dt.float32
    bf16 = mybir.dt.bfloat16
    Alu = mybir.AluOpType
    Act = mybir.ActivationFunctionType

    nc = bacc.Bacc("TRN2", target_bir_lowering=False, debug=False,
                   num_devices=N_CORES)

    img = nc.dram_tensor("img", [C_IMG, L, W], bf16, kind="ExternalInput").ap()
    xa = nc.dram_tensor("xa", [KA, CELLS], bf16, kind="ExternalInput").ap()
    xb = nc.dram_tensor("xb", [KB, CELLS], bf16, kind="ExternalInput").ap()
    bta_d = nc.dram_tensor("bta", [KA, 9], bf16, kind="ExternalInput").ap()
    btb_d = nc.dram_tensor("btb", [KB, 9], bf16, kind="ExternalInput").ap()
    w3d = nc.dram_tensor("w3", [C_IMG], f32, kind="ExternalInput").ap()
    # shift matrices: [128, 3*128 + 3] = Sh0 | Sh1 | Sh2 | Shb1(1) | Shb2(2)
    shd = nc.dram_tensor("shmats", [128, 387], bf16, kind="ExternalInput").ap()
    emgd = nc.dram_tensor("emg", [L, 96], bf16, kind="ExternalInput").ap()
    rowfixd = nc.dram_tensor("rowfix", [R, 1], f32, kind="ExternalInput").ap()
    colfix0d = nc.dram_tensor("colfix0", [R, 1], f32, kind="ExternalInput").ap()
    colfix1d = nc.dram_tensor("colfix1", [R, 1], f32, kind="ExternalInput").ap()
    out = nc.dram_tensor("out", [C_IMG, R, W], f32, kind="ExternalOutput").ap()

    with tile.TileContext(nc) as tc:
        with (
            tc.tile_pool(name="persist", bufs=1) as pp,
            tc.tile_pool(name="work", bufs=3) as wp,
            tc.tile_pool(name="imgp", bufs=3) as ip,
            tc.tile_pool(name="psum", bufs=1, space="PSUM") as psp,
            tc.tile_pool(name="pg", bufs=1, space="PSUM") as pgp,
            tc.tile_pool(name="plg", bufs=1, space="PSUM") as plgp,
            tc.tile_pool(name="ppts", bufs=2, space="PSUM") as psb,
        ):
            # ---- persistent small tensors ----
            bta_t = pp.tile([KA, 9], bf16, tag="bta")
            nc.sync.dma_start(out=bta_t[:], in_=bta_d[:])
            btb_t = pp.tile([KB, 9], bf16, tag="btb")
            nc.sync.dma_start(out=btb_t[:], in_=btb_d[:])
            w3f_t = pp.tile([C_IMG // 2, 2], f32, tag="w3f")
            nc.sync.dma_start(out=w3f_t[:],
                              in_=w3d[:].rearrange("(hh c) -> c hh", hh=2))
            w3_t = pp.tile([C_IMG // 2, 2], bf16, tag="w3")
            nc.vector.tensor_copy(out=w3_t[:], in_=w3f_t[:])
            sh_t = pp.tile([128, 387], bf16, tag="shmats")
            nc.sync.dma_start(out=sh_t[:], in_=shd[:])
            emg_t = pp.tile([L, 96], bf16, tag="emg")
            nc.sync.dma_start(out=emg_t[:], in_=emgd[:])
            ones_t = pp.tile([1, 128], bf16, tag="ones")
            nc.vector.memset(ones_t[:], 1.0)
            RB = R // 2
            rowfix_b, colfix0_b, colfix1_b = [], [], []
            for b in range(2):
                rt = pp.tile([RB, 1], f32, tag=f"rowfix{b}", name=f"rowfix{b}")
                nc.sync.dma_start(out=rt[:], in_=rowfixd[b * RB:(b + 1) * RB, :])
                rowfix_b.append(rt)
                c0t = pp.tile([RB, 1], f32, tag=f"colfix0{b}", name=f"cf0{b}")
                nc.sync.dma_start(out=c0t[:],
                                  in_=colfix0d[b * RB:(b + 1) * RB, :])
                colfix0_b.append(c0t)
                c1t = pp.tile([RB, 1], f32, tag=f"colfix1{b}", name=f"cf1{b}")
                nc.sync.dma_start(out=c1t[:],
                                  in_=colfix1d[b * RB:(b + 1) * RB, :])
                colfix1_b.append(c1t)

            # preload the sigmoid ACT table off the critical path
            warm = pp.tile([1, 2], f32, tag="warm")
            nc.vector.memset(warm[:], 0.0)
            nc.scalar.activation(warm[:, 0:1], warm[:, 1:2], Act.Sigmoid,
                                 bias=0.0, scale=1.0)

            gmap = pp.tile([L, WP], bf16, tag="gmap")
            nc.vector.memset(gmap[:], 0.0)
            T_sb = pp.tile([128, BLOCKS * 9], bf16, tag="Tsb")
            att_b = [pp.tile([RB, W], f32, tag=f"att{b}", name=f"att{b}")
                     for b in range(2)]
            attbf_b = [pp.tile([RB, W], bf16, tag=f"attbf{b}",
                               name=f"attbf{b}") for b in range(2)]
            # persistent bf16 image cache: [2 c-halves][128, L*W]
            img_bf = [pp.tile([128, L * W], bf16, tag=f"imgbf{hh}",
                              name=f"imgbf{hh}")
                      for hh in range(2)]

            # ---- two row-phases: each runs its half of stage B groups and
            # stage C rows, then assembles attention for its 16-row band and
            # streams the output writes overlapped with the next phase ----
            T3 = T_sb[:].rearrange("p (h x) -> p h x", x=NT * 9)
            segs = ((0, 512), (512, 192))
            BR = R // 2
            GRP_PH = (range(0, 9), range(9, NGRP))
            RC_PH = (range(0, 9), range(9, L // 2))
            for ph in range(2):
                b0r = ph * BR

                # -- stage B groups of this phase --
                for g in GRP_PH[ph]:
                    c0 = g * GRP * 128
                    xat = wp.tile([KA, GRP * 128], bf16, tag="xa")
                    nc.sync.dma_start(out=xat[:],
                                      in_=xa[:, c0:c0 + GRP * 128])
                    xbt = wp.tile([KB, GRP * 128], bf16, tag="xb")
                    nc.sync.dma_start(out=xbt[:],
                                      in_=xb[:, c0:c0 + GRP * 128])
                    tpsum = psb.tile([128, GRP * 9], f32, tag="pts")
                    for bl in range(GRP):
                        po = bl * 9
                        cx = bl * 128
                        nc.tensor.matmul(tpsum[:, po:po + 9],
                                         xbt[:, cx:cx + 128], btb_t[:],
                                         start=True, stop=False)
                        nc.tensor.matmul(tpsum[:, po:po + 9],
                                         xat[:, cx:cx + 128], bta_t[:],
                                         start=False, stop=True)
                    nc.scalar.copy(
                        out=T_sb[:, g * GRP * 9:(g + 1) * GRP * 9],
                        in_=tpsum[:])

                # -- stage C rows of this phase (img lands in bf16 cache) --
                for rc in RC_PH[ph]:
                    imgt = []
                    for hh in range(2):
                        ib = img_bf[hh][:, 2 * rc * W:(2 * rc + 2) * W]
                        nc.scalar.dma_start(
                            out=ib,
                            in_=img[hh * 128:(hh + 1) * 128,
                                    2 * rc:2 * rc + 2, :].rearrange(
                                        "c r w -> c (r w)"))
                        imgt.append(ib)
                    gstage = wp.tile([1, 2 * W], bf16, tag="gstage")
                    for off, n in ((0, 512), (512, 512), (1024, 384)):
                        gp = pgp.tile([1, 512], f32, tag="gp")
                        for hh in range(2):
                            nc.tensor.matmul(
                                gp[:, 0:n],
                                w3_t[:, hh:hh + 1],
                                imgt[hh][:, off:off + n],
                                start=(hh == 0), stop=(hh == 1))
                        nc.scalar.copy(out=gstage[:, off:off + n],
                                       in_=gp[:, 0:n])
                    for r01 in range(2):
                        nc.scalar.dma_start(
                            out=gmap[2 * rc + r01:2 * rc + r01 + 1, 1:1 + W],
                            in_=gstage[:, r01 * W:(r01 + 1) * W])

                # -- dy-reduce for this band (GpSimd) --
                U = [[None] * 3 for _ in range(NT)]
                for m in range(NT):
                    for d in range(3):
                        u = pp.tile([128, BR], bf16, tag=f"u{m}{d}{ph}",
                                    name=f"u{m}{d}{ph}")
                        o = lambda dy: m * 9 + 3 * dy + d
                        nc.gpsimd.tensor_tensor(
                            out=u[:], in0=T3[:, b0r:b0r + BR, o(0)],
                            in1=T3[:, b0r + 1:b0r + 1 + BR, o(1)], op=Alu.add)
                        nc.gpsimd.tensor_tensor(
                            out=u[:], in0=u[:],
                            in1=T3[:, b0r + 2:b0r + 2 + BR, o(2)],
                            op=Alu.add)
                        U[m][d] = u

                # -- logit assembly for this band on PE --
                # band 0 contracts over only its 18 gmap rows (rows 18+ not
                # written yet); band 1 uses all 34 (rows 0..15 have zero
                # weights in its emg column slice). Matmul operand slices
                # must start at partition 0.
                ge = emg_t[0:18, :] if ph == 0 else emg_t[:]
                gm = gmap[0:18, :] if ph == 0 else gmap[:]
                lg = plgp.tile([BR, W], f32, tag="lg")
                for gi, (off, n) in enumerate(segs):
                    nc.tensor.matmul(
                        lg[:, off:off + n],
                        ge[:, 0 + b0r:0 + b0r + BR],
                        gm[:, 0:W][:, off:off + n], start=True, stop=False)
                    nc.tensor.matmul(
                        lg[:, off:off + n],
                        ge[:, 32 + b0r:32 + b0r + BR],
                        gm[:, 1:1 + W][:, off:off + n],
                        start=False, stop=False)
                for m in range(NT):
                    mc = m * 128
                    wm = 128 if m < NT - 1 else W - 128 * (NT - 1)
                    for d in range(3):
                        nc.tensor.matmul(
                            lg[:, mc:mc + wm], U[m][d][:],
                            sh_t[:, d * 128:d * 128 + wm],
                            start=False, stop=False)
                    if m < NT - 1:
                        nc.tensor.matmul(
                            lg[:, mc + 127:mc + 128], U[m + 1][1][:],
                            sh_t[:, 384:385], start=False, stop=False)
                        nc.tensor.matmul(
                            lg[:, mc + 126:mc + 128], U[m + 1][2][:],
                            sh_t[:, 385:387], start=False, stop=False)
                for gi, (off, n) in enumerate(segs):
                    nc.tensor.matmul(
                        lg[:, off:off + n],
                        ge[:, 64 + b0r:64 + b0r + BR],
                        gm[:, 2:2 + W][:, off:off + n],
                        start=False, stop=True)
                nc.vector.tensor_tensor(
                    out=lg[:, 0:1], in0=lg[:, 0:1],
                    in1=colfix0_b[ph][:, 0:1], op=Alu.add)
                nc.vector.tensor_tensor(
                    out=lg[:, W - 1:W], in0=lg[:, W - 1:W],
                    in1=colfix1_b[ph][:, 0:1], op=Alu.add)
                # rowfix carries C_all + sp_b + row-edge constants
                nc.scalar.activation(att_b[ph][:], lg[:], Act.Sigmoid,
                                     bias=rowfix_b[ph][:, 0:1], scale=1.0)
                nc.vector.tensor_copy(out=attbf_b[ph][:], in_=att_b[ph][:])

                # -- broadcast multiply + store for this band --
                for rc in range(ph * (BR // 2), (ph + 1) * (BR // 2)):
                    a1p = wp.tile([1, 2 * W], bf16, tag="a1p")
                    for r01 in range(2):
                        rl = 2 * rc + r01 - b0r
                        nc.scalar.dma_start(
                            out=a1p[:, r01 * W:(r01 + 1) * W],
                            in_=attbf_b[ph][rl:rl + 1, :])
                    attb = psp.tile([128, 2 * W], f32, tag="big")
                    for off, n in ((0, 512), (512, 512), (1024, 384)):
                        nc.tensor.matmul(
                            attb[:, off:off + n],
                            ones_t[:],
                            a1p[:, off:off + n],
                            start=True, stop=True)
                    attb_sb = wp.tile([128, 2 * W], bf16, tag="attbsb")
                    nc.scalar.copy(out=attb_sb[:], in_=attb[:])
                    for hh in range(2):
                        ib = img_bf[hh][:, (2 * rc + 1) * W:(2 * rc + 3) * W]
                        ot = ip.tile([128, 2 * W], f32, tag="out")
                        nc.vector.tensor_tensor(out=ot[:], in0=ib,
                                                in1=attb_sb[:], op=Alu.mult)
                        nc.sync.dma_start(
                            out=out[hh * 128:(hh + 1) * 128,
                                    2 * rc:2 * rc + 2, :].rearrange(
                                        "c r w -> c (r w)"),
                            in_=ot[:])

    nc.compile()
    return nc


def _prepare(inputs):
    """Host-side fold + shard. Returns (b3, in_maps)."""
    fold = _fold_weights(inputs)
    cc, S = fold["cc"], fold["S"]

    grids = [np.asarray(inputs[f"img_grid_{i}"]) for i in range(3)]
    feats = [np.asarray(inputs[f"voxel_feat_{i}"]) for i in range(3)]
    coords = [np.asarray(inputs[f"voxel_coord_{i}"]) for i in range(3)]
    img_feat = np.asarray(inputs["img_feat"])

    # stacked (set0|set1|set2-tail) and set2-main folded B matrices, bf16
    bta = np.concatenate(
        [fold["B"][0].T, fold["B"][1].T, fold["B"][2].T[128:]], axis=0)
    btb = fold["B"][2].T[:128]

    # shift matrices Sh_d[c, f] = 1 iff c == f + d; boundary picks
    sh = np.zeros((128, 387), BF16)
    for d in range(3):
        for f in range(128 - d):
            sh[f + d, d * 128 + f] = 1.0
    sh[0, 384] = 1.0              # Shb1: col f=127 <- U_{m+1,1}[0]
    sh[0, 385] = 1.0              # Shb2: col f=126 <- U_{m+1,2}[0]
    sh[1, 386] = 1.0              # Shb2: col f=127 <- U_{m+1,2}[1]

    # gated tap matrices: emg[p+dy, dx*32+p] = S[dy*3+dx] (f32 base;
    # per-core rowmask is folded in before the bf16 cast)
    emg_base = np.zeros((L, 96), np.float32)
    for dx in range(3):
        for dy in range(3):
            for p in range(R):
                emg_base[p + dy, dx * 32 + p] = np.float32(S[dy * 3 + dx])

    # per-set x (channels x tokens), f32 -> bf16 once
    xs_bf = []
    for i in range(3):
        xs_bf.append(np.ascontiguousarray(np.concatenate(
            [feats[i], coords[i]], axis=1).T).astype(BF16))

    in_maps = []
    for c in range(N_CORES):
        lo = R * c - 1
        m = {}
        slab = np.zeros((C_IMG, L, W), BF16)
        g0, g1 = max(lo, 0), min(lo + L, H)
        slab[:, g0 - lo:g1 - lo, :] = img_feat[:, g0:g1, :].astype(BF16)
        m["img"] = slab

        xa = np.zeros((KA, CELLS), BF16)
        xb = np.zeros((KB, CELLS), BF16)
        offs = [0, CH[0] + 3, None]
        for i in range(3):
            rows = grids[i][:, 1]
            sel = np.nonzero((rows >= lo) & (rows < lo + L))[0]
            if not len(sel):
                continue
            hl = rows[sel].astype(np.int64) - lo
            wl = grids[i][sel, 0].astype(np.int64) + 1
            cell = hl * WS + wl
            xi = xs_bf[i][:, sel]
            if i < 2:
                xa[offs[i]:offs[i] + xi.shape[0], cell] = xi
            else:
                xb[:, cell] = xi[:128]
                xa[KA - 3:, cell] = xi[128:]
        m["xa"] = xa
        m["xb"] = xb
        m["bta"] = np.ascontiguousarray(bta).astype(BF16)
        m["btb"] = np.ascontiguousarray(btb).astype(BF16)
        m["w3"] = fold["w3"].astype(np.float32)
        m["shmats"] = sh
        rowmask = np.zeros((L, 1), np.float32)
        rowmask[g0 - lo:g1 - lo] = 1.0
        m["emg"] = (emg_base * rowmask).astype(BF16)
        # rowfix: C_all + sp_b + row-edge constants (used as sigmoid bias)
        rowfix = np.full((R, 1), fold["C_all"] + fold["spb"], np.float64)
        colfix0 = np.full((R, 1), -(cc[0] + cc[3] + cc[6]))
        colfix1 = np.full((R, 1), -(cc[2] + cc[5] + cc[8]))
        for hloc in range(R):
            g = R * c + hloc
            if g == 0:
                rowfix[hloc] += -(cc[0] + cc[1] + cc[2])
                colfix0[hloc] += cc[0]
                colfix1[hloc] += cc[2]
            if g == H - 1:
                rowfix[hloc] += -(cc[6] + cc[7] + cc[8])
                colfix0[hloc] += cc[6]
                colfix1[hloc] += cc[8]
        m["rowfix"] = rowfix.astype(np.float32)
        m["colfix0"] = colfix0.astype(np.float32)
        m["colfix1"] = colfix1.astype(np.float32)
        in_maps.append(m)
    return fold["b3"], in_maps


def kernel(**inputs):
    global LAST_RESULT
    from concourse.bass_utils import run_bass_kernel_spmd

    b3, in_maps = _prepare(inputs)
    nc = _build_program(b3)
    res = run_bass_kernel_spmd(nc, in_maps, core_ids=list(range(N_CORES)))
    LAST_RESULT = res
    out = np.concatenate(
        [res.results[c]["out"] for c in range(N_CORES)], axis=1)
    return np.ascontiguousarray(out.astype(np.float32))
